# revision 14
# baseline (speedup 1.0000x reference)
"""Multi-head attention block (B=32,S=512,D=768,H=12) on 8 TRN2 NeuronCores.

Sharding: data-parallel over batch (4 batches/core), weights replicated,
no collectives. Host pre-transposes x and the weight matrices so the
device kernel is a pure matmul pipeline (no on-chip transposes).

v2 schedule: head-PAIR processing with row-tiled concurrent scores
matmuls (even/odd heads live at partitions 0:64 / 64:128 of the same
yT chunk, so the two K=64 matmuls execute concurrently on disjoint
PE row-groups), and the attention stream (scores -> exp -> av -> norm)
is woven instruction-by-instruction into the qkv(b)/proj(b-1) matmul
stream so the Scalar-engine exp work (~28us/batch) overlaps the
PE-dense projection phases instead of serializing the attention phase.

  per core (4 batches), all matmul operands bf16, fp32 PSUM accum:
    yT[o,t]  = Wqkv xT for q,k rows (ACT per-partition bias on the
             psum->sbuf copy)
    v[t,o]   natural, bias added during the interleave copy; stored as
             [v_h | 1] per head so row 64 of the av output is the
             softmax denominator for free.
    per pair p (heads 2p,2p+1): scoresT[s,t] chunks via 2 row-tiled
             matmuls (K=64 each) into one [128,1024] psum; exp on ACT
             [128,1024] ops, scale folded.  av per head in its own
             psum pool; sums row stacking + waved reciprocal + gpsimd
             partition-broadcast + DVE multiply as before.
    out[t,:] = avT^T WpT + combo; DVE tensor-tensor add + DMA out.
"""

import sys

if "/opt/trn_rl_repo" not in sys.path:
    sys.path.insert(0, "/opt/trn_rl_repo")

from contextlib import ExitStack

import numpy as np

import concourse.tile as tile
from concourse import bacc, mybir
from concourse.bass_utils import run_bass_kernel_spmd

B, S, D = 32, 512, 768
H, HD = 12, 64
SCALE = HD**-0.5
NCORES = 8
NB = B // NCORES  # batches per core
P = 128
TCH = S // P  # token chunks per batch
DCH = D // P  # d chunks
QKC = 2 * D // P  # o-chunks holding q,k
NHALF = D // 2  # 384: N-tile for v/proj matmuls
NPAIR = H // 2
F32 = mybir.dt.float32
BF16 = mybir.dt.bfloat16
EXP = mybir.ActivationFunctionType.Exp


def build_nc():
    nc = bacc.Bacc(None, target_bir_lowering=False, debug=False)
    xT = nc.declare_dram_parameter("xT", [NB, D, S], BF16, isOutput=False)
    wqkvT = nc.declare_dram_parameter("wqkvT", [D, 3 * D], BF16, isOutput=False)
    wpT = nc.declare_dram_parameter("wpT", [D, D], BF16, isOutput=False)
    bqkv = nc.declare_dram_parameter("bqkv", [3 * D], F32, isOutput=False)
    combo = nc.declare_dram_parameter("combo", [D], BF16, isOutput=False)
    bv16 = nc.declare_dram_parameter("bv16", [D], BF16, isOutput=False)
    out = nc.declare_dram_parameter("out", [NB, S, D], F32, isOutput=True)

    WAVE = 6

    with ExitStack() as ctx:
        tc = ctx.enter_context(tile.TileContext(nc))
        wp = ctx.enter_context(tc.tile_pool(name="weights", bufs=1))
        sb = ctx.enter_context(tc.tile_pool(name="work", bufs=1))
        ps = ctx.enter_context(tc.tile_pool(name="psum", bufs=1, space="PSUM"))

        # ---- persistent weight/const tiles (DMAs emitted below, after
        # the batch-0 x load, in need-order) ----
        wq_t = [
            wp.tile([P, 3 * D], BF16, name=f"wqkvT{d}", tag=f"wqkvT{d}")
            for d in range(DCH)
        ]
        bcolall = wp.tile([P, QKC], F32, name="bcolall", tag="bcolall")
        bcols = [bcolall[:, c : c + 1] for c in range(QKC)]
        bvrow = wp.tile([1, D], BF16, name="bvrow", tag="bvrow")
        bvb = wp.tile([P, D], BF16, name="bvb", tag="bvb")
        ones = wp.tile([1, P], BF16, name="ones", tag="ones")
        wp_t = [
            wp.tile([P, D], BF16, name=f"wpT{d}", tag=f"wpT{d}")
            for d in range(DCH)
        ]
        comborow = wp.tile([1, D], BF16, name="comborow", tag="comborow")
        cbb = wp.tile([P, D], BF16, name="cbb", tag="cbb")

        def emit_x_load(b):
            xt = []
            for d in range(DCH):
                t = sb.tile([P, S], BF16, name=f"xT_b{b}_{d}", tag=f"xT{d}", bufs=2)
                nc.gpsimd.dma_start(out=t, in_=xT[b, d * P : (d + 1) * P, :])
                xt.append(t)
            return xt

        def emit_yT_chunk(b, xt, c):
            pt = ps.tile([P, S], F32, name=f"yTps_b{b}_{c}", tag="mm", bufs=2)
            for d in range(DCH):
                nc.tensor.matmul(
                    out=pt,
                    lhsT=wq_t[d][:, c * P : (c + 1) * P],
                    rhs=xt[d],
                    start=(d == 0),
                    stop=(d == DCH - 1),
                )
            st = sb.tile([P, S], BF16, name=f"yT_b{b}_{c}", tag=f"yT{c}", bufs=2)
            nc.scalar.activation(
                st, pt, mybir.ActivationFunctionType.Identity, bias=bcols[c]
            )
            return st

        def emit_v_tile(b, xt, ti):
            vtile = sb.tile(
                [P, H * (HD + 1)], BF16, name=f"v_b{b}_{ti}", tag=f"v{ti}", bufs=2
            )
            nc.vector.memset(
                vtile.rearrange("p (h k) -> p h k", k=HD + 1)[:, :, HD : HD + 1],
                1.0,
            )
            for half in range(2):
                pv = ps.tile(
                    [P, NHALF], F32, name=f"vps_b{b}_{ti}_{half}", tag="mm", bufs=2
                )
                o0 = 2 * D + half * NHALF
                for d in range(DCH):
                    nc.tensor.matmul(
                        out=pv,
                        lhsT=xt[d][:, ti * P : (ti + 1) * P],
                        rhs=wq_t[d][:, o0 : o0 + NHALF],
                        start=(d == 0),
                        stop=(d == DCH - 1),
                    )
                nc.vector.tensor_tensor(
                    out=vtile.rearrange("p (h k) -> p h k", k=HD + 1)[
                        :, 6 * half : 6 * (half + 1), 0:HD
                    ],
                    in0=pv.rearrange("p (h k) -> p h k", k=HD),
                    in1=bvb[:, half * NHALF : (half + 1) * NHALF].rearrange(
                        "p (h k) -> p h k", k=HD
                    ),
                    op=mybir.AluOpType.add,
                )
            return vtile

        def emit_sc(b, p, yt, ets, jj):
            """Two scoresT chunks (j in {2*jj, 2*jj+1}) for head pair p.

            Each chunk is one [128,1024] psum: cols 0:512 = even head
            (K=64 row-tile at rows 0-63), cols 512:1024 = odd head
            (rows 64-127); the two matmuls run concurrently on disjoint
            row-groups.  One exp ACT op covers both heads' chunk.
            """
            for j in (2 * jj, 2 * jj + 1):
                pt = ps.tile(
                    [P, 2 * S], F32, name=f"sc_b{b}_p{p}_j{j}", tag="sc", bufs=2
                )
                nc.tensor.matmul(
                    out=pt[:, 0:S],
                    lhsT=yt[6 + p][0:HD, j * P : (j + 1) * P],
                    rhs=yt[p][0:HD, :],
                    start=True,
                    stop=True,
                )
                nc.tensor.matmul(
                    out=pt[:, S : 2 * S],
                    lhsT=yt[6 + p][HD:P, j * P : (j + 1) * P],
                    rhs=yt[p][HD:P, :],
                    start=True,
                    stop=True,
                )
                et = sb.tile(
                    [P, 2 * S], BF16, name=f"expT_b{b}_p{p}_j{j}", tag="expT",
                    bufs=8,
                )
                nc.scalar.activation(et, pt, EXP, scale=SCALE)
                ets.append(et)

        def post_head(b, h, avt, state):
            """avsb copy, sums-row stacking, and (at wave ends) the
            reciprocal + broadcast + normalize chain.  Identical to the
            baseline per-head normalization machinery."""
            wi = h - state["wave_start"]
            avsb = state["avsbs"][h]
            nc.sync.dma_start(
                out=state["stacked"][32 * wi : 32 * wi + 4, :],
                in_=avsb[HD : HD + 1, :],
            )
            if h in state["wave_ends"]:
                w0 = state["wave_start"]
                recw = sb.tile([P, P], F32, name=f"recw_b{b}_h{h}", tag="recw",
                               bufs=2)
                nc.vector.reciprocal(recw, state["stacked"])
                for hh in range(w0, h + 1):
                    wj = hh - w0
                    rrow = sb.tile([1, S], BF16, name=f"rrow_b{b}_h{hh}",
                                   tag="rrow", bufs=2 * WAVE)
                    nc.gpsimd.dma_start(
                        out=rrow, in_=recw[32 * wj : 32 * wj + 4, :]
                    )
                    bc = sb.tile([HD, S], BF16, name=f"bc_b{b}_h{hh}", tag="bc",
                                 bufs=WAVE + 1)
                    nc.gpsimd.partition_broadcast(bc, rrow)
                    c = hh // 2
                    src_av = state["avsbs"][hh]
                    if hh % 2 == 0:
                        nc.vector.tensor_mul(avt[c][:HD, :], src_av[:HD, :], bc)
                    else:
                        tmp = sb.tile([HD, S], BF16, name=f"avtmp_b{b}_h{hh}",
                                      tag="avtmp", bufs=4)
                        nc.vector.tensor_mul(tmp, src_av[:HD, :], bc)
                        nc.sync.dma_start(out=avt[c][HD : 2 * HD, :], in_=tmp)
                state["stacked"] = sb.tile(
                    [P, P], BF16, name=f"stk_b{b}_h{h}", tag="stacked", bufs=2
                )
                nc.vector.memset(state["stacked"], 1.0)
                state["wave_start"] = h + 1

        def emit_av_pair(b, p, vt, ets, avt, state):
            """av matmuls + normalization bookkeeping for heads 2p, 2p+1."""
            for h in (2 * p, 2 * p + 1):
                col0 = 0 if h % 2 == 0 else S
                pav = ps.tile([HD + 1, S], F32, name=f"av_b{b}_h{h}", tag="av",
                              bufs=2)
                for j in range(TCH):
                    nc.tensor.matmul(
                        out=pav,
                        lhsT=vt[j][:, h * (HD + 1) : (h + 1) * (HD + 1)],
                        rhs=ets[4 * p + j][:, col0 : col0 + S],
                        start=(j == 0),
                        stop=(j == TCH - 1),
                    )
                avsb = sb.tile([HD + 1, S], BF16, name=f"avsb_b{b}_h{h}",
                               tag="avsb", bufs=8)
                nc.vector.tensor_copy(avsb, pav)
                state["avsbs"].append(avsb)
                post_head(b, h, avt, state)

        def emit_proj_ti(b, avt, ti):
            ft = sb.tile([P, D], F32, name=f"fin_b{b}_{ti}", tag="fin", bufs=3)
            for half in range(2):
                pf = ps.tile(
                    [P, NHALF], F32, name=f"fps_b{b}_{ti}_{half}", tag="mm",
                    bufs=2,
                )
                for d in range(DCH):
                    nc.tensor.matmul(
                        out=pf,
                        lhsT=avt[d][:, ti * P : (ti + 1) * P],
                        rhs=wp_t[d][:, half * NHALF : (half + 1) * NHALF],
                        start=(d == 0),
                        stop=(d == DCH - 1),
                    )
                nc.vector.tensor_tensor(
                    out=ft[:, half * NHALF : (half + 1) * NHALF],
                    in0=pf,
                    in1=cbb[:, half * NHALF : (half + 1) * NHALF],
                    op=mybir.AluOpType.add,
                )
            nc.sync.dma_start(out=out[b, ti * P : (ti + 1) * P, :], in_=ft)

        def new_state(b):
            st = {
                "avsbs": [],
                "wave_start": 0,
                "wave_ends": {2, 5, 8, 10, 11} if b == NB - 1 else {2, 5, 8, 11},
                "stacked": sb.tile([P, P], BF16, name=f"stk_b{b}_init",
                                   tag="stacked", bufs=2),
            }
            nc.vector.memset(st["stacked"], 1.0)
            return st

        # ---- main schedule ----
        # Cycle b emits qkv(b) units woven with attn(b) pairs 0-2 (each
        # pair's av lagging its scores by >=2 units so the ACT exp work
        # overlaps PE-dense qkv/proj matmuls), proj(b-1), and the tail
        # of attn(b-1) (av of pair 3, scores+av of pairs 4,5) so the
        # exp-gated matmuls at a batch's end always have independent
        # qkv work behind them in the PE queue.
        # x(0) first: it gates the first matmul and rides the gpsimd
        # queue, which must not sit behind broadcast ops.  The tiny
        # bias rows go out next (they gate broadcasts / yT copies), and
        # bulk weight traffic is spread across four issue queues since
        # each dma_start costs ~0.6us of issue time on its engine.
        xt = emit_x_load(0)
        nc.gpsimd.dma_start(
            out=bcolall, in_=bqkv[: 2 * D].rearrange("(c p) -> p c", p=P)
        )
        nc.sync.dma_start(out=bvrow, in_=bv16.rearrange("(o f) -> o f", o=1))
        nc.sync.dma_start(out=comborow, in_=combo.rearrange("(o f) -> o f", o=1))
        nc.gpsimd.partition_broadcast(bvb, bvrow)
        nc.gpsimd.partition_broadcast(cbb, comborow)
        nc.vector.memset(ones, 1.0)
        # startup weights in need-order (startup is transfer-bound at
        # ~358GB/s): chunk-0 columns first (gate the first matmul),
        # then the q remainder, k, v, proj.  The b=0 schedule below
        # consumes q-chunks first so compute overlaps the k/v streams.
        for d in range(DCH):
            nc.scalar.dma_start(
                out=wq_t[d][:, 0:P], in_=wqkvT[d * P : (d + 1) * P, 0:P]
            )
        for d in range(DCH):
            nc.sync.dma_start(
                out=wq_t[d][:, P:D], in_=wqkvT[d * P : (d + 1) * P, P:D]
            )
        for d in range(DCH):
            nc.scalar.dma_start(
                out=wq_t[d][:, D : 2 * D],
                in_=wqkvT[d * P : (d + 1) * P, D : 2 * D],
            )
        for d in range(DCH):
            eng = nc.sync if d % 2 == 0 else nc.scalar
            eng.dma_start(
                out=wq_t[d][:, 2 * D :], in_=wqkvT[d * P : (d + 1) * P, 2 * D :]
            )
        for d in range(DCH):
            eng = nc.sync if d % 2 == 0 else nc.scalar
            eng.dma_start(out=wp_t[d], in_=wpT[d * P : (d + 1) * P, :])

        prev = None  # attn state of batch b-1: dict(avt, vt, ets, state)
        for b in range(NB):
            xtb = xt
            yt = [None] * QKC

            if prev is None:
                # b=0 prologue: q-chunks first (their weights land
                # first), k and v streams overlap the q matmuls.
                for c in range(6):
                    yt[c] = emit_yT_chunk(b, xtb, c)
                yt[6] = emit_yT_chunk(b, xtb, 6)
                yt[7] = emit_yT_chunk(b, xtb, 7)
                ets = []
                emit_sc(b, 0, yt, ets, 0)
                yt[8] = emit_yT_chunk(b, xtb, 8)
                yt[9] = emit_yT_chunk(b, xtb, 9)
                emit_sc(b, 0, yt, ets, 1)
                yt[10] = emit_yT_chunk(b, xtb, 10)
                yt[11] = emit_yT_chunk(b, xtb, 11)
                emit_sc(b, 1, yt, ets, 0)
                vt = [emit_v_tile(b, xtb, 0), emit_v_tile(b, xtb, 1)]
                emit_sc(b, 1, yt, ets, 1)
                vt.append(emit_v_tile(b, xtb, 2))
                vt.append(emit_v_tile(b, xtb, 3))
                xt = emit_x_load(b + 1)
                avt = [
                    sb.tile([P, S], BF16, name=f"avT_b{b}_{c}", tag=f"avT{c}",
                            bufs=2)
                    for c in range(DCH)
                ]
                state = new_state(b)
                emit_av_pair(b, 0, vt, ets, avt, state)
                emit_sc(b, 2, yt, ets, 0)
                emit_av_pair(b, 1, vt, ets, avt, state)
                emit_sc(b, 2, yt, ets, 1)
                emit_av_pair(b, 2, vt, ets, avt, state)
                emit_sc(b, 3, yt, ets, 0)
                emit_sc(b, 3, yt, ets, 1)
                prev = {"avt": avt, "vt": vt, "ets": ets, "state": state,
                        "yt": yt}
                continue

            yt[0] = emit_yT_chunk(b, xtb, 0)
            yt[6] = emit_yT_chunk(b, xtb, 6)
            emit_av_pair(b - 1, 3, prev["vt"], prev["ets"], prev["avt"],
                         prev["state"])
            emit_sc(b - 1, 4, prev["yt"], prev["ets"], 0)
            yt[1] = emit_yT_chunk(b, xtb, 1)
            yt[7] = emit_yT_chunk(b, xtb, 7)
            emit_sc(b - 1, 4, prev["yt"], prev["ets"], 1)
            yt[2] = emit_yT_chunk(b, xtb, 2)
            yt[8] = emit_yT_chunk(b, xtb, 8)
            emit_av_pair(b - 1, 4, prev["vt"], prev["ets"], prev["avt"],
                         prev["state"])
            emit_sc(b - 1, 5, prev["yt"], prev["ets"], 0)
            yt[3] = emit_yT_chunk(b, xtb, 3)
            yt[9] = emit_yT_chunk(b, xtb, 9)
            emit_sc(b - 1, 5, prev["yt"], prev["ets"], 1)
            yt[4] = emit_yT_chunk(b, xtb, 4)
            yt[10] = emit_yT_chunk(b, xtb, 10)
            # last av-pair of b-1 emitted before the v tiles so its
            # normalization chain owns the DVE/GpSimd queues and
            # avt(b-1) is complete well before proj(b-1) needs it.
            emit_av_pair(b - 1, 5, prev["vt"], prev["ets"], prev["avt"],
                         prev["state"])
            vt = [emit_v_tile(b, xtb, 0), emit_v_tile(b, xtb, 1)]
            ets = []
            emit_sc(b, 0, yt, ets, 0)
            if b + 1 < NB:
                xt = emit_x_load(b + 1)
            vt.append(emit_v_tile(b, xtb, 2))
            emit_sc(b, 0, yt, ets, 1)
            vt.append(emit_v_tile(b, xtb, 3))
            emit_proj_ti(b - 1, prev["avt"], 0)
            emit_sc(b, 1, yt, ets, 0)
            avt = [
                sb.tile([P, S], BF16, name=f"avT_b{b}_{c}", tag=f"avT{c}", bufs=2)
                for c in range(DCH)
            ]
            state = new_state(b)
            emit_av_pair(b, 0, vt, ets, avt, state)
            emit_proj_ti(b - 1, prev["avt"], 1)
            emit_sc(b, 1, yt, ets, 1)
            yt[5] = emit_yT_chunk(b, xtb, 5)
            yt[11] = emit_yT_chunk(b, xtb, 11)
            emit_proj_ti(b - 1, prev["avt"], 2)
            emit_sc(b, 2, yt, ets, 0)
            emit_av_pair(b, 1, vt, ets, avt, state)
            emit_proj_ti(b - 1, prev["avt"], 3)
            emit_sc(b, 2, yt, ets, 1)
            emit_av_pair(b, 2, vt, ets, avt, state)
            emit_sc(b, 3, yt, ets, 0)
            emit_sc(b, 3, yt, ets, 1)
            prev = {"avt": avt, "vt": vt, "ets": ets, "state": state,
                    "yt": yt}

        # epilogue: attn tail of the last batch + its projection, with
        # proj ti=0/1 accumulation split by d so the d<=2 matmuls run
        # while the last pairs' exps and normalization drain.
        b = NB - 1
        emit_av_pair(b, 3, prev["vt"], prev["ets"], prev["avt"], prev["state"])
        emit_sc(b, 4, prev["yt"], prev["ets"], 0)
        emit_sc(b, 4, prev["yt"], prev["ets"], 1)
        emit_av_pair(b, 4, prev["vt"], prev["ets"], prev["avt"], prev["state"])
        emit_sc(b, 5, prev["yt"], prev["ets"], 0)
        emit_sc(b, 5, prev["yt"], prev["ets"], 1)
        avt = prev["avt"]
        groups = []
        for k, (ti, half) in enumerate([(0, 0), (0, 1), (1, 0), (1, 1)]):
            pf = ps.tile(
                [P, NHALF], F32, name=f"fps_b{b}_{ti}_{half}",
                tag=["sc", "sc", "mm", "mm"][k], bufs=2,
            )
            groups.append((pf, ti, half))
        for pf, ti, half in groups:
            for d in range(3):
                nc.tensor.matmul(
                    out=pf,
                    lhsT=avt[d][:, ti * P : (ti + 1) * P],
                    rhs=wp_t[d][:, half * NHALF : (half + 1) * NHALF],
                    start=(d == 0),
                    stop=False,
                )
        emit_av_pair(b, 5, prev["vt"], prev["ets"], prev["avt"], prev["state"])
        # ti=2 rides the av psum pool (free once the last pair's CASTs
        # drain); stage d-levels by avt readiness so the proj stream
        # never fully blocks on the final normalization waves.
        for half in range(2):
            pf = ps.tile([P, NHALF], F32, name=f"fps_b{b}_2_{half}", tag="av",
                         bufs=2)
            groups.append((pf, 2, half))
        for pf, ti, half in groups:
            d0 = 0 if ti == 2 else 3
            for d in range(d0, 4):
                nc.tensor.matmul(
                    out=pf,
                    lhsT=avt[d][:, ti * P : (ti + 1) * P],
                    rhs=wp_t[d][:, half * NHALF : (half + 1) * NHALF],
                    start=(d == 0),
                    stop=False,
                )
        fts = {}
        for pf, ti, half in groups:
            for d in range(4, DCH):
                nc.tensor.matmul(
                    out=pf,
                    lhsT=avt[d][:, ti * P : (ti + 1) * P],
                    rhs=wp_t[d][:, half * NHALF : (half + 1) * NHALF],
                    start=False,
                    stop=(d == DCH - 1),
                )
            if ti not in fts:
                fts[ti] = sb.tile([P, D], F32, name=f"fin_b{b}_{ti}", tag="fin",
                                  bufs=3)
            nc.vector.tensor_tensor(
                out=fts[ti][:, half * NHALF : (half + 1) * NHALF],
                in0=pf,
                in1=cbb[:, half * NHALF : (half + 1) * NHALF],
                op=mybir.AluOpType.add,
            )
            if half == 1:
                nc.sync.dma_start(
                    out=out[b, ti * P : (ti + 1) * P, :], in_=fts[ti]
                )
        emit_proj_ti(b, avt, 3)

    nc.compile()
    return nc


_CACHE = {}


def _get_nc():
    if "nc" not in _CACHE:
        _CACHE["nc"] = build_nc()
    return _CACHE["nc"]


def _prepare_in_maps(x, qkv_w, qkv_b, proj_w, proj_b):
    x = np.asarray(x, dtype=np.float32)
    qkv_w = np.asarray(qkv_w, dtype=np.float32)
    qkv_b = np.asarray(qkv_b, dtype=np.float32)
    proj_w = np.asarray(proj_w, dtype=np.float32)
    proj_b = np.asarray(proj_b, dtype=np.float32)
    import ml_dtypes

    bf16 = ml_dtypes.bfloat16
    wqkvT = np.ascontiguousarray(qkv_w.T).astype(bf16)
    wpT = np.ascontiguousarray(proj_w.T).astype(bf16)
    combo = proj_b.astype(bf16)  # v-bias flows through softmax via bvrow
    bv16 = qkv_b[2 * D :].astype(bf16)
    in_maps = []
    for c in range(NCORES):
        xs = x[c * NB : (c + 1) * NB]
        xTs = np.ascontiguousarray(xs.transpose(0, 2, 1)).astype(bf16)
        in_maps.append(
            {
                "xT": xTs,
                "wqkvT": wqkvT,
                "wpT": wpT,
                "bqkv": qkv_b,
                "combo": combo,
                "bv16": bv16,
            }
        )
    return in_maps


def kernel(x, qkv_w, qkv_b, proj_w, proj_b):
    nc = _get_nc()
    in_maps = _prepare_in_maps(x, qkv_w, qkv_b, proj_w, proj_b)
    res = run_bass_kernel_spmd(nc, in_maps, core_ids=list(range(NCORES)))
    return np.concatenate([res.results[i]["out"] for i in range(NCORES)], axis=0)
